# revision 25
# baseline (speedup 1.0000x reference)
"""Trainium2 Bass kernel for nn_ConvolutionalSelfAttention.

The reference network collapses algebraically. Per image b (Xt = batch[b]
viewed [C, HW], c-major):
  K_t = key_w @ Xt + key_b          [C, HW]
  Q_t = query_w @ Xt + query_b      [C, HW]
  v   = value_w @ Xt + value_b      [HW]
  rk[n] = 1/||K_t[:,n]||, rq[m] = 1/||Q_t[:,m]||
  E[n,m] = exp(rk[n] rq[m] (K_t[:,n] . Q_t[:,m]))       (full 1024x1024 Gram)
  V[m] = (sum_n v[n] E[n,m]) / (sum_n E[n,m])
  P[c,m] = Xt[c,m] V[m]
  out[b,c,i,j] = 3x3 valid box-sum of P over the spatial grid

Sharding: data-parallel over batch (16 images over 8 cores, 2 each).

v3: inputs arrive bf16 (half the DMA); the Gram runs as fp8e4 DoubleRow
matmuls (K=256 contracted per column at 1 col/cycle, 2x bf16 rate), with
the dual-fp8 slab layouts LDWEIGHTS requires; the 3x3 box-sum runs on
DVE (flat bf16 adds, 2x) + GpSimd instead of identity matmuls; numer and
denom stay on the PE (M=2 f32r matmul over E row-blocks). Image 1's
gram/V/conv pipeline is split into column halves so its V round-trip and
conv overlap the second half of its own gram. Row-blocks of E use the
permuted order n = p*8 + j so every partition-transpose bounce through
DRAM moves contiguous lines per partition.
"""
import os
import numpy as np
import ml_dtypes

os.environ.setdefault("BASS_NEVER_TRACE", "1")

import contextlib

import concourse.bass as bass
import concourse.bacc as bacc
import concourse.tile as tile
from concourse import mybir
from concourse.bass_utils import run_bass_kernel_spmd

F32 = mybir.dt.float32
F32R = mybir.dt.float32r
BF16 = mybir.dt.bfloat16
FP8 = mybir.dt.float8e4
AF = mybir.ActivationFunctionType
ALU = mybir.AluOpType
DR = mybir.MatmulPerfMode.DoubleRow

B, C, H, W = 16, 256, 32, 32
HW = H * W            # 1024
CH = CW = 30
NF = CH * CW          # 900
NCORES = 8
BL = B // NCORES      # images per core
NCH = C // 128        # channel chunks
NNJ = HW // 128       # position chunks


def _ap(t, extra_off, pattern):
    return bass.AP(tensor=t.tensor, offset=t.offset + extra_off,
                   ap=[list(x) for x in pattern])


def build_program():
    nc = bacc.Bacc("TRN2", target_bir_lowering=False, debug=False,
                   num_devices=NCORES)
    # walrus's lower_act places activation-table loads; bacc's pre-placed
    # loads produce NEFFs this runtime refuses to load.
    nc.insert_act_table_loads = lambda: None

    def din(name, shape, dt):
        return nc.dram_tensor(name, list(shape), dt, kind="ExternalInput").ap()

    x_d = din("x", (BL, C, HW), BF16)
    wall_d = din("wall", (C, 2 * C + 2), BF16)   # [query_w.T | key_w.T | value_w.T]
    ball_d = din("ball", (128, 2 * NCH), F32)    # [bk | bq]
    bv_d = din("bv", (1, 2), F32)

    out_d = nc.dram_tensor("out", [BL, C, NF], F32, kind="ExternalOutput").ap()

    with tile.TileContext(nc) as tc:
        with contextlib.ExitStack() as ctx:
            consts = ctx.enter_context(tc.tile_pool(name="consts", bufs=1))
            sbuf = ctx.enter_context(tc.tile_pool(name="sbuf", bufs=2))
            epool = ctx.enter_context(tc.tile_pool(name="epool", bufs=4))
            big = ctx.enter_context(tc.tile_pool(name="big", bufs=3, space="PSUM"))
            small = ctx.enter_context(tc.tile_pool(name="small", bufs=1, space="PSUM"))
            dramp = ctx.enter_context(tc.tile_pool(name="dramp", bufs=2, space="DRAM"))

            # ---------------- constants ----------------
            wall_t = consts.tile([128, NCH, 2 * C + 2], BF16, tag="wall", name="wall")
            wallv = wall_d.rearrange("(kc p) m -> p kc m", p=128)
            nc.scalar.dma_start(out=wall_t[:, :, 0:C], in_=wallv[:, :, 0:C])
            wq_t = wall_t[:, :, 0:C]
            wk_t = wall_t[:, :, C:2 * C]
            wv_t = wall_t[:, :, 2 * C:2 * C + 2]
            ball_t = consts.tile([128, 2 * NCH], F32, tag="ball", name="ball")
            bk_t = ball_t[:, 0:NCH]
            bq_t = ball_t[:, NCH:2 * NCH]
            bv_t = consts.tile([128, 1], F32, tag="bv", name="bv")
            ones_b = consts.tile([128, 1], BF16, tag="ones_b", name="ones_b")
            nc.vector.memset(ones_b, 1.0)
            ones8 = consts.tile([128, NCH, 1], FP8, tag="ones8", name="ones8")
            nc.vector.memset(ones8, 1.0)
            # fp8 copies of the Q/K weights in dual-fp8 slab layout:
            # w8[p, qk, mc, kc, m] = wall[p, kc, qk*C + mc*128 + m]
            w8 = consts.tile([128, 2, NCH, NCH, 128], FP8, tag="w8", name="w8")

            def conv_w8():
                for qk in range(2):
                    for mc in range(NCH):
                        nc.vector.tensor_copy(
                            w8[:, qk, mc, :, :],
                            wall_t[:, :, qk * C + mc * 128:
                                   qk * C + (mc + 1) * 128])

            def load_rest_consts():
                nc.scalar.dma_start(out=wall_t[:, :, C:], in_=wallv[:, :, C:])
                nc.sync.dma_start(out=ball_t, in_=ball_d)
                nc.sync.dma_start(out=bv_t, in_=_ap(bv_d, 0, [[0, 128], [1, 1]]))

            def warmup():
                # ones_b comes from a memset (no DMA dependency), so these
                # N=1 matmuls start immediately and keep the PE busy through
                # the input-load window -> HAM unthrottles before the first
                # projection matmul.
                pw = small.tile([128, HW], F32, tag="small", name="warm")
                for _ in range(12):
                    nc.tensor.matmul(pw[0:1, 0:1], ones_b, ones_b,
                                     start=True, stop=True)

            # ---------------- per-b state ----------------
            xs, qts, sqbs, kns, sk8s, qns = {}, {}, {}, {}, {}, {}
            rkts, rq_bcs, V_bcs, v1s, pssqs, pnds = {}, {}, {}, {}, {}, {}
            d_ssqQ, d_ssqK, d_v, d_nd, d_V = {}, {}, {}, {}, {}
            convst = {}

            def load_x(b):
                xs[b] = sbuf.tile([128, NCH, HW], BF16, tag="x", name="x")
                xv = x_d[b].rearrange("(kc p) m -> p kc m", p=128)
                engs = [nc.sync, nc.sync] if b == 0 else [nc.scalar, nc.scalar]
                for kc in range(NCH):
                    engs[kc].dma_start(out=xs[b][:, kc, :], in_=xv[:, kc, :])

            x8s = {}

            def conv_x8(b):
                # x8[p, nt, kc, n] fp8, (kc, n) slab contiguous per nt
                x8 = sbuf.tile([128, 2, NCH, 512], FP8, tag="x8", name="x8")
                x8s[b] = x8
                for kc in range(NCH):
                    nc.vector.tensor_copy(
                        x8[:, :, kc, :],
                        xs[b][:, kc, :].rearrange("p (nt n) -> p nt n", nt=2))

            def mm_proj(psum, qk, mc, b):
                for nt in range(2):
                    nc.tensor.matmul(
                        psum[:, nt * 512:(nt + 1) * 512],
                        w8[:, qk, mc, :, :],
                        x8s[b][:, nt, :, :],
                        start=True, stop=True, perf_mode=DR)

            def projQ(b):
                qts[b] = sbuf.tile([128, NCH, HW], BF16, tag="qt", name="qt")
                sqb = sbuf.tile([128, NCH, HW], BF16, tag="sqb", name="sqb")
                sqbs[b] = sqb
                for mc in range(NCH):
                    pp = big.tile([128, HW], F32, tag="big", name="big")
                    mm_proj(pp, 0, mc, b)
                    nc.vector.tensor_scalar_add(
                        qts[b][:, mc, :], pp, bq_t[:, mc:mc + 1])
                    nc.vector.tensor_tensor(
                        out=sqb[:, mc, :], in0=qts[b][:, mc, :],
                        in1=qts[b][:, mc, :], op=ALU.mult)
                pssq = small.tile([128, HW], F32, tag="small", name="small")
                pssqs[b] = pssq
                for nt in range(2):
                    for kc in range(NCH):
                        nc.tensor.matmul(
                            pssq[0:1, nt * 512:(nt + 1) * 512],
                            ones_b,
                            sqb[:, kc, nt * 512:(nt + 1) * 512],
                            start=(kc == 0), stop=(kc == NCH - 1))
                s_sq = sbuf.tile([1, HW], F32, tag="s_sqQ", name="s_sqQ")
                nc.scalar.copy(s_sq, pssq[0:1, :])
                d_ssqQ[b] = dramp.tile([1, HW], F32, tag="d_ssqQ", name="d_ssqQ")
                nc.sync.dma_start(out=d_ssqQ[b], in_=s_sq)

            def rqchain(b):
                # ssqQ[p, j] = d_ssqQ[0, p*8+j] -> rq = exp(-0.5 ln(.))
                t = sbuf.tile([128, NNJ], F32, tag="rqt", name="rqt")
                nc.sync.dma_start(
                    out=t, in_=_ap(d_ssqQ[b], 0, [[NNJ, 128], [1, NNJ]]))
                nc.scalar.activation(t, t, AF.Ln)
                t_bf = sbuf.tile([128, NNJ], BF16, tag="rqb", name="rqb")
                nc.scalar.activation(t_bf, t, AF.Exp, scale=-0.5)
                d_rq = dramp.tile([1, HW], BF16, tag="d_rq", name="d_rq")
                nc.sync.dma_start(
                    out=_ap(d_rq, 0, [[NNJ, 128], [1, NNJ]]), in_=t_bf)
                rq_bc = sbuf.tile([128, HW], BF16, tag="rq_bc", name="rq_bc")
                nc.sync.dma_start(
                    out=rq_bc, in_=_ap(d_rq, 0, [[0, 128], [1, HW]]))
                rq_bcs[b] = rq_bc

            def projKV(b):
                # kn2[p, j, kc, q] = K[kc*128+p, n=q*8+j] + bk: per row-block
                # j the (kc, q) slab is contiguous, as dual-fp8 LDWEIGHTS
                # requires. Bias-adds on Act (strided src is free there).
                kn2 = sbuf.tile([128, NNJ, NCH, 128], FP8, tag="kn", name="kn")
                kns[b] = kn2
                sk8 = sbuf.tile([128, NCH, HW], FP8, tag="sk8", name="sk8")
                sk8s[b] = sk8
                for mc in range(NCH):
                    pp = big.tile([128, HW], F32, tag="big", name="big")
                    mm_proj(pp, 1, mc, b)
                    ppv = pp.rearrange("p (q j) -> p j q", j=NNJ)
                    nc.scalar.activation(
                        kn2[:, :, mc, :], ppv, AF.Identity,
                        bias=bk_t[:, mc:mc + 1])
                    nc.gpsimd.tensor_tensor(
                        out=sk8[:, mc, :].rearrange("p (q j) -> p q j", j=NNJ),
                        in0=kn2[:, :, mc, :].rearrange("p nj q -> p q nj"),
                        in1=kn2[:, :, mc, :].rearrange("p nj q -> p q nj"),
                        op=ALU.mult)
                pssq = pssqs[b]
                # K-side column norms: plain fp8 matmul, kc-accumulated
                for nt in range(2):
                    for kc in range(NCH):
                        nc.tensor.matmul(
                            pssq[32:33, nt * 512:(nt + 1) * 512],
                            ones8[:, kc, :],
                            sk8[:, kc, nt * 512:(nt + 1) * 512],
                            start=(kc == 0), stop=(kc == NCH - 1))
                # v projection (M=1): own PSUM tile, dst partition base 0
                pv = big.tile([128, HW], F32, tag="big", name="big")
                for kc in range(NCH):
                    for nt in range(2):
                        nc.tensor.matmul(
                            pv[0:1, nt * 512:(nt + 1) * 512],
                            wv_t[:, kc, 0:1],
                            xs[b][:, kc, nt * 512:(nt + 1) * 512],
                            start=(kc == 0), stop=(kc == NCH - 1))
                s_sk = sbuf.tile([1, HW], F32, tag="s_sk", name="s_sk")
                nc.scalar.copy(s_sk, pssq[32:33, :])
                d_ssqK[b] = dramp.tile([1, HW], F32, tag="d_ssqK", name="d_ssqK")
                nc.gpsimd.dma_start(out=d_ssqK[b], in_=s_sk)
                s_v = sbuf.tile([1, HW], F32, tag="s_v", name="s_v")
                nc.scalar.copy(s_v, pv[0:1, :])
                d_v[b] = dramp.tile([1, HW], F32, tag="d_v", name="d_v")
                nc.scalar.dma_start(out=d_v[b], in_=s_v)

            def rkchain(b):
                t = sbuf.tile([128, NNJ], F32, tag="rkt", name="rkt")
                nc.gpsimd.dma_start(
                    out=t, in_=_ap(d_ssqK[b], 0, [[NNJ, 128], [1, NNJ]]))
                nc.scalar.activation(t, t, AF.Ln)
                nc.scalar.activation(t, t, AF.Exp, scale=-0.5)
                rkts[b] = t

            def vprep(b):
                v_sb = sbuf.tile([128, NNJ], F32, tag="v_sb", name="v_sb")
                nc.scalar.dma_start(
                    out=v_sb, in_=_ap(d_v[b], 0, [[NNJ, 128], [1, NNJ]]))
                # col 0 = v+bias, col 32 = 1 -> numer on PSUM row 0, denom
                # on row 32 (engine ops need quadrant-aligned partition base)
                v1f = sbuf.tile([128, NNJ, 33], F32, tag="v1f", name="v1f")
                nc.vector.memset(v1f, 0.0)
                nc.vector.tensor_scalar_add(v1f[:, :, 0], v_sb, bv_t[:, 0:1])
                nc.vector.memset(v1f[:, :, 32], 1.0)
                v1r = sbuf.tile([128, NNJ, 33], F32R, tag="v1r", name="v1r")
                nc.vector.tensor_copy(v1r, v1f)
                v1s[b] = v1r

            def qnorm(b):
                # qn2[p, nt, kc, n]: per nt the (kc, n) slab is contiguous
                qn2 = sbuf.tile([128, 2, NCH, 512], FP8, tag="qn", name="qn")
                qns[b] = qn2
                for mc in range(NCH):
                    nc.vector.tensor_tensor(
                        out=qn2[:, :, mc, :],
                        in0=qts[b][:, mc, :].rearrange(
                            "p (nt n) -> p nt n", nt=2),
                        in1=rq_bcs[b].rearrange("p (nt n) -> p nt n", nt=2),
                        op=ALU.mult)

            def gram(b):
                kn2 = kns[b]
                qn2 = qns[b]
                pnd = small.tile([128, HW], F32, tag="small", name="small")
                pnds[b] = pnd
                pgs, es = [None] * NNJ, [None] * NNJ

                def gram_chunk(nj):
                    pg = big.tile([128, HW], F32, tag="big", name="big")
                    pgs[nj] = pg
                    for nt in range(2):
                        nc.tensor.matmul(
                            pg[:, nt * 512:(nt + 1) * 512],
                            kn2[:, nj, :, :],
                            qn2[:, nt, :, :],
                            start=True, stop=True, perf_mode=DR)

                def exp_chunk(nj):
                    e = epool.tile([128, HW], F32R, tag="e", name="e")
                    es[nj] = e
                    nc.scalar.activation(
                        e, pgs[nj], AF.Exp, scale=rkts[b][:, nj:nj + 1])

                def numer_chunk(nj):
                    for nt in range(2):
                        nc.tensor.matmul(
                            pnd[0:33, nt * 512:(nt + 1) * 512],
                            v1s[b][:, nj, :],
                            es[nj][:, nt * 512:(nt + 1) * 512],
                            start=(nj == 0), stop=(nj == NNJ - 1))

                LAG = 3
                for nj in range(NNJ):
                    gram_chunk(nj)
                    exp_chunk(nj)
                    if nj >= LAG:
                        numer_chunk(nj - LAG)
                for nj in range(NNJ - LAG, NNJ):
                    numer_chunk(nj)

            def vcalc(b):
                # V row computed in place on partitions 0/1 of pnd, then one
                # bounce through DRAM for the partition-broadcast.
                # 1/denom = exp(-ln(denom)) on the (tail-idle) Act engine
                lden = sbuf.tile([1, HW], F32, tag="lden", name="lden")
                nc.scalar.activation(lden, pnds[b][32:33, :], AF.Ln)
                rden = sbuf.tile([1, HW], F32, tag="rden", name="rden")
                nc.scalar.activation(rden, lden, AF.Exp, scale=-1.0)
                V_row = sbuf.tile([1, HW], BF16, tag="V_row", name="V_row")
                nc.vector.tensor_tensor(
                    out=V_row, in0=pnds[b][0:1, :], in1=rden, op=ALU.mult)
                d_V[b] = dramp.tile([1, HW], BF16, tag="d_V", name="d_V")
                nc.sync.dma_start(out=d_V[b], in_=V_row)
                V_bcs[b] = sbuf.tile([128, HW], BF16, tag="V_bc", name="V_bc")
                for h, eng in ((0, nc.sync), (1, nc.sync)):
                    eng.dma_start(
                        out=V_bcs[b][:, h * 512:(h + 1) * 512],
                        in_=_ap(d_V[b], h * 512, [[0, 128], [1, 512]]))

            def conv(b, rows, vadd_eng):
                # out rows i0..i1-1 need P columns i0*32..(i1+2)*32. All sums
                # are flat 1-D adds (garbage in the w=30,31 lanes is skipped
                # by the strided output DMA). vadd_eng: engine for the two
                # vertical adds ("gpsimd" to overlap, "vector" for the tail).
                i0, i1 = rows
                m0, m1 = i0 * W, (i1 + 2) * W
                x_f = xs[b]
                st = convst.setdefault(b, {})
                for mc in range(NCH):
                    veng = nc.vector
                    stm = st.setdefault(mc, {})
                    if "p_sb" not in stm:
                        stm["p_sb"] = sbuf.tile([128, HW], BF16, tag="p_sb",
                                                name="p_sb")
                        stm["s1"] = sbuf.tile([128, HW], BF16, tag="s1",
                                              name="s1")
                        nc.vector.memset(stm["s1"][:, 510:512], 0.0)
                        nc.vector.memset(stm["s1"][:, HW - 2:HW], 0.0)
                        stm["t2"] = sbuf.tile([128, HW], BF16, tag="t2",
                                              name="t2")
                        stm["o"] = sbuf.tile([128, CH, CW], F32, tag="o_sb",
                                             name="o_sb")
                    p_sb, s1, t2, o = (stm["p_sb"], stm["s1"], stm["t2"],
                                       stm["o"])
                    nc.vector.tensor_tensor(
                        out=p_sb[:, m0:m1], in0=x_f[:, mc, m0:m1],
                        in1=V_bcs[b][:, m0:m1], op=ALU.mult)
                    nc.vector.tensor_tensor(
                        out=s1[:, m0:m1 - 2], in0=p_sb[:, m0:m1 - 2],
                        in1=p_sb[:, m0 + 1:m1 - 1], op=ALU.add)
                    nc.vector.tensor_tensor(
                        out=s1[:, m0:m1 - 2], in0=s1[:, m0:m1 - 2],
                        in1=p_sb[:, m0 + 2:m1], op=ALU.add)
                    veng.tensor_tensor(
                        out=t2[:, m0:m1 - 64], in0=s1[:, m0:m1 - 64],
                        in1=s1[:, m0 + 32:m1 - 32], op=ALU.add)
                    t2v = t2.rearrange("p (h w) -> p h w", h=H)
                    s1v = s1.rearrange("p (h w) -> p h w", h=H)
                    veng.tensor_tensor(
                        out=o[:, i0:i1, :], in0=t2v[:, i0:i1, 0:CW],
                        in1=s1v[:, i0 + 2:i1 + 2, 0:CW], op=ALU.add)
                    nc.sync.dma_start(
                        out=_ap(out_d[b], mc * 128 * NF + i0 * CW,
                                [[NF, 128], [1, (i1 - i0) * CW]]),
                        in_=o[:, i0:i1, :].rearrange("p h w -> p (h w)"))

            # ---------------- emission schedule ----------------
            warmup()
            load_x(0)
            load_rest_consts()
            conv_w8()
            load_x(1)
            conv_x8(0)
            projQ(0)
            rqchain(0)
            projKV(0)
            rkchain(0)
            vprep(0)
            qnorm(0)
            conv_x8(1)
            projQ(1)
            rqchain(1)
            projKV(1)
            rkchain(1)
            vprep(1)
            qnorm(1)
            gram(0)
            vcalc(0)
            conv(0, rows=(0, CH), vadd_eng="vector")
            gram(1)
            vcalc(1)
            conv(1, rows=(0, CH), vadd_eng="vector")

    nc.compile()
    return nc


_CACHE = {}


def _get_program():
    if "nc" not in _CACHE:
        _CACHE["nc"] = build_program()
    return _CACHE["nc"]


def make_in_maps(batch, key_w, key_b, query_w, query_b, value_w, value_b):
    bf16 = ml_dtypes.bfloat16
    wall = np.zeros((C, 2 * C + 2), np.float32)
    wall[:, 0:C] = query_w.T
    wall[:, C:2 * C] = key_w.T
    wall[:, 2 * C] = value_w[0]
    ball = np.zeros((128, 2 * NCH), np.float32)
    ball[:, 0:NCH] = key_b.reshape(NCH, 128).T
    ball[:, NCH:2 * NCH] = query_b.reshape(NCH, 128).T
    bv = np.zeros((1, 2), np.float32)
    bv[0, 0] = value_b[0]
    in_maps = []
    for i in range(NCORES):
        xb = batch[i * BL:(i + 1) * BL].reshape(BL, C, HW)
        in_maps.append({
            "x": np.ascontiguousarray(xb.astype(bf16)),
            "wall": wall.astype(bf16), "ball": ball, "bv": bv,
        })
    return in_maps


def kernel(batch, key_w, key_b, query_w, query_b, value_w, value_b,
           local_indices=None, **_ignored):
    batch = np.ascontiguousarray(np.asarray(batch, np.float32))
    args = [np.asarray(a, np.float32) for a in
            (key_w, key_b, query_w, query_b, value_w, value_b)]
    nc = _get_program()
    in_maps = make_in_maps(batch, *args)
    res = run_bass_kernel_spmd(nc, in_maps, list(range(NCORES)))
    outs = [np.asarray(r["out"], np.float32) for r in res.results]
    return np.concatenate(outs, axis=0).reshape(B, C, CH, CW)
